# revision 1
# baseline (speedup 1.0000x reference)
"""TRN2 Bass kernel for nn_GAT_73950746902569 — instruction-count-minimized v2.

Backend charges ~40-90us per instruction nearly independent of data size, so
the design maximizes work per instruction: multi-row indirect gathers, wide
strided vector ops over superblocks (7 blocks x 128 targets), xbar bf16
transposes, per-pair edge-feature tables, and balanced edge packing.
"""
import numpy as np
import ml_dtypes

import concourse.bass as bass
import concourse.bacc as bacc
import concourse.mybir as mybir
import concourse.tile as tile
from concourse.bass_utils import run_bass_kernel_spmd

N, E, B = 100000, 200000, 4096
HID, EDIM, HEADS, L, NCLS = 256, 64, 8, 4, 3
M = 8
NPC = N // M            # 12500
NB = 98
NPAD = NB * 128         # 12544
SBW = 7                 # blocks per superblock
NSB = NB // SBW         # 14
GPC = B // M            # 512
BPAD = 4224
NPAIR = 484             # 22*22
P = 128

F32 = mybir.dt.float32
BF16 = mybir.dt.bfloat16
I32 = mybir.dt.int32
ALU = mybir.AluOpType
ACTF = mybir.ActivationFunctionType
AX = mybir.AxisListType.X

_cache = {}


def _bits(a):
    """[n] uint -> [n,8] f32 bits MSB-first."""
    return (((np.asarray(a)[:, None] >> np.arange(7, -1, -1)) & 1)
            .astype(np.float32))


def _bits_rows(a):
    """[n,k] -> [n,8k] f32 MSB-first per byte."""
    a = np.asarray(a)
    bits = ((a[:, :, None] >> np.arange(7, -1, -1)) & 1)
    return bits.reshape(a.shape[0], -1).astype(np.float32)


def _rep(v, n=128):
    v = np.asarray(v, np.float32)
    return np.broadcast_to(v[None, :], (n, v.shape[-1])).copy()


def _pack_core(deg):
    """Assign NPC local nodes to (block, lane), packing per-superblock edge
    counts into as few 128-slots as possible. Returns pos[NPC] and per-(sb,j)
    edge counts (blocks ordered by count desc; short block last in sb 13)."""
    pos = np.empty(NPC, np.int64)
    keb = np.zeros((NSB, SBW), np.int64)
    for g in range(NSB):
        lo, hi = g * 896, min((g + 1) * 896, NPC)
        nodes = np.arange(lo, hi)
        d = deg[lo:hi]
        order = np.argsort(-d, kind="stable")
        nodes, d = nodes[order], d[order]
        nreal = hi - lo
        tot = int(d.sum())
        caps = [128] * SBW
        if nreal < 896:
            caps[SBW - 1] = nreal - 128 * (SBW - 1)
        rest = max(tot - 1664, 1)
        targets = [384] + [256] * 5 + [rest]
        used = np.zeros(len(nodes), bool)
        sums = [0] * SBW
        takes = [[] for _ in range(SBW)]
        # phase 1: big nodes toward targets, stop short of overshoot
        for j in range(SBW):
            for i in range(len(nodes)):
                if used[i] or len(takes[j]) == caps[j]:
                    continue
                if sums[j] + d[i] <= targets[j]:
                    used[i] = True
                    takes[j].append(i)
                    sums[j] += d[i]
        # phase 2: place leftovers (desc) where ceil-headroom allows
        for i in range(len(nodes)):
            if used[i]:
                continue
            best, bestcost = None, None
            for j in range(SBW):
                if len(takes[j]) == caps[j]:
                    continue
                K = (sums[j] + 127) // 128 if sums[j] > 0 else 0
                head = K * 128 - sums[j]
                cost = 0 if d[i] <= head else (d[i] - head + 127) // 128
                tie = sums[j]
                if best is None or (cost, tie) < bestcost:
                    best, bestcost = j, (cost, tie)
            used[i] = True
            takes[best].append(i)
            sums[best] += d[i]
        order_j = sorted(range(SBW), key=lambda j: -sums[j])
        if nreal < 896:
            order_j = [j for j in order_j if caps[j] == 128] + \
                      [j for j in order_j if caps[j] != 128]
        for newj, oldj in enumerate(order_j):
            take = np.array(takes[oldj], np.int64)
            b = g * SBW + newj
            pos[nodes[take]] = b * 128 + np.arange(len(take))
            keb[g, newj] = sums[oldj]
    return pos, keb


def host_prep(inputs):
    x = np.asarray(inputs["x"])
    edge_index = np.asarray(inputs["edge_index"])
    edge_attr = np.asarray(inputs["edge_attr"])
    batch = np.asarray(inputs["batch"])

    src, tgt = edge_index[0].astype(np.int64), edge_index[1].astype(np.int64)
    pair = (edge_attr[:, 0] * 22 + edge_attr[:, 1]).astype(np.int64)

    # ---- weight-derived tables (shared across cores) ----
    atom_emb = np.asarray(inputs["atom_emb"], np.float32)        # [120,128]
    alw = np.asarray(inputs["atom_lin_w"], np.float32)           # [56,128]
    alb = np.asarray(inputs["atom_lin_b"], np.float32)           # [128]
    edge_emb = np.asarray(inputs["edge_emb"], np.float32)        # [22,64]
    elw = np.asarray(inputs["edge_lin_w"], np.float32)           # [8,64]
    elb = np.asarray(inputs["edge_lin_b"], np.float32)           # [64]
    lin_l_w = np.asarray(inputs["lin_l_w"], np.float32)
    lin_r_w = np.asarray(inputs["lin_r_w"], np.float32)
    lin_e_w = np.asarray(inputs["lin_e_w"], np.float32)

    a0g, a1g = np.meshgrid(np.arange(22), np.arange(22), indexing="ij")
    ef_pairs = np.concatenate(
        [edge_emb[a0g.ravel()], _bits(a1g.ravel()) @ elw + elb],
        axis=1).astype(np.float32)                               # [484,128]
    eft = np.zeros((NPAIR, 132), np.float32)
    eft[:, :128] = ef_pairs
    eft[:, 128] = 1.0
    eetab_pairs = np.stack(
        [ef_pairs @ lin_e_w[l] for l in range(L)]).astype(np.float32)

    W = {}
    W["eetab_pairs"] = eetab_pairs                              # [L,484,256]
    W["wcat"] = np.stack([
        np.stack([np.concatenate([lin_l_w[l, 128 * h:128 * (h + 1)],
                                  lin_r_w[l, 128 * h:128 * (h + 1)]], axis=1)
                  for h in range(2)]) for l in range(L)
    ]).astype(ml_dtypes.bfloat16)                               # [L,2,128,512]
    W["xlr_b"] = np.stack([
        _rep(np.concatenate([np.asarray(inputs["lin_l_b"])[l],
                             np.asarray(inputs["lin_r_b"])[l]]))
        for l in range(L)])                                     # [L,128,512]
    W["lew"] = lin_e_w.astype(ml_dtypes.bfloat16)               # [L,128,256]
    W["att_rep"] = np.stack([_rep(np.asarray(inputs["att"])[l])
                             for l in range(L)])
    W["convb_rep"] = np.stack([_rep(np.asarray(inputs["conv_b"])[l])
                               for l in range(L)])
    W["bng"] = np.asarray(inputs["bn_g"], np.float32)[:, None, :]
    W["bnb"] = np.asarray(inputs["bn_b"], np.float32)[:, None, :]
    aemb_pad = np.zeros((128, 128), np.float32)
    aemb_pad[:120] = atom_emb
    W["aemb_pad"] = aemb_pad
    W["alw"] = alw
    W["alb_col"] = alb[:, None].astype(np.float32)              # [128,1]
    W["iota"] = np.broadcast_to(np.arange(128, dtype=np.float32)[None, :],
                                (128, 128)).copy()
    mask97 = np.zeros((128, 1), np.float32)
    W["mask97"] = mask97  # filled per-core? same for all: lanes < 84
    mask97[:NPC - 97 * 128] = 1.0
    for k in ("w1", "w2", "w3", "w4"):
        W[k] = np.asarray(inputs[k], np.float32).astype(ml_dtypes.bfloat16)
    for k, wd in (("b1", 1024), ("b2", 1024), ("b3", 512), ("b4", NCLS)):
        W[k + "_rep"] = _rep(np.asarray(inputs[k]))

    # ---- loop_attr (input-derived) ----
    deg_all = np.bincount(tgt, minlength=N)
    order = np.argsort(tgt, kind="stable")
    ef_e = ef_pairs[pair[order]]                                # [E,128] f32
    starts = np.searchsorted(tgt[order], np.arange(N + 1))
    nonempty = deg_all > 0
    la = np.zeros((N, 128), np.float32)
    la[nonempty] = np.add.reduceat(ef_e, starts[:-1][nonempty], axis=0)
    la /= np.maximum(deg_all, 1)[:, None]
    gcnt = np.bincount(np.asarray(batch, np.int64), minlength=B)
    rcg_all = (1.0 / np.maximum(gcnt, 1)).astype(np.float32)

    # ---- per-core packing ----
    pos_all = np.empty(N, np.int64)
    kebs = []
    for c in range(M):
        sl = slice(c * NPC, (c + 1) * NPC)
        pos, keb = _pack_core(deg_all[sl])
        pos_all[sl] = pos
        kebs.append(keb)
    Ktab = np.maximum.reduce([(k + 127) // 128 for k in kebs])   # [NSB,SBW]
    gpad = (np.arange(N) // NPC) * NPAD + pos_all                # global padded row

    SE = int(Ktab.sum())
    S = SE + NB
    # global slot col layout: per sb: edge slots (block j asc, k asc), then
    # 7 self slots. Edge-slot-only index for trel/st.
    sb_e0 = np.zeros(NSB + 1, np.int64)    # edge-slot base per sb
    for g in range(NSB):
        sb_e0[g + 1] = sb_e0[g] + Ktab[g].sum()

    lfT_h = np.zeros((M, 128, NPAD), ml_dtypes.bfloat16)
    rcg_h = np.zeros((M, 128, GPC // 128), np.float32)
    for c in range(M):
        sl = slice(c * NPC, (c + 1) * NPC)
        laT = np.zeros((128, NPAD), np.float32)
        laT[:, pos_all[sl]] = la[sl].T
        lfT_h[c] = laT.astype(ml_dtypes.bfloat16)
        rcg_h[c] = rcg_all[c * GPC:(c + 1) * GPC].reshape(
            GPC // 128, 128).T

    src_idx = np.zeros((M, 128, S), np.int32)
    tgt_idx = np.zeros((M, 128, S), np.int32)
    ee_idx = np.zeros((M, 128, S), np.int32)
    trel = np.full((M, 128, SE), 200.0, np.float32)
    x0row = np.zeros((M, 1, NPAD), np.float32)
    bitsT = np.zeros((M, 56, NPAD), np.float32)
    brel = np.full((M, 128, NB), 200.0, np.float32)
    pidx = np.zeros((M, 128, NB), np.int32)

    for c in range(M):
        sl = slice(c * NPC, (c + 1) * NPC)
        pos = pos_all[sl]
        # node-indexed uploads in padded layout
        x0row[c, 0, pos] = x[sl][:, 0].astype(np.float32)
        bitsT[c][:, pos] = _bits_rows(x[sl][:, 1:8]).T
        bc = batch[sl]
        for b in range(NB):
            lanes = np.where(pos // 128 == b)[0]
            lane_of = pos[lanes] % 128
            gb = int(bc[lanes].min()) if len(lanes) else 0
            assert len(lanes) == 0 or int(bc[lanes].max()) - gb < 128
            brel[c, lane_of, b] = bc[lanes] - gb
            pidx[c, :, b] = gb + np.arange(128)
        # edges of this core grouped by target block
        em = (tgt >= c * NPC) & (tgt < (c + 1) * NPC)
        et, es, ep = tgt[em] - c * NPC, src[em], pair[em]
        epos = pos[et]
        eb = epos // 128
        order = np.argsort(eb, kind="stable")
        et, es, ep, epos, eb = et[order], es[order], ep[order], epos[order], eb[order]
        starts = np.searchsorted(eb, np.arange(NB + 1))
        for g in range(NSB):
            col = sb_e0[g]
            for j in range(SBW):
                b = g * SBW + j
                e0, e1 = starts[b], starts[b + 1]
                cnt = e1 - e0
                K = int(Ktab[g, j])
                assert cnt <= K * 128, (c, g, j, cnt, K)
                for k in range(K):
                    lo = e0 + k * 128
                    hi = min(e1, lo + 128)
                    mlen = max(hi - lo, 0)
                    if mlen > 0:
                        src_idx[c, :mlen, col] = gpad[es[lo:hi]]
                        tgt_idx[c, :mlen, col] = epos[lo:hi]
                        ee_idx[c, :mlen, col] = ep[lo:hi]
                        trel[c, :mlen, col] = (epos[lo:hi] % 128).astype(np.float32)
                    col += 1
    colmap_edge = np.zeros(SE, np.int64)
    colmap_self = np.zeros(NB, np.int64)
    cc = 0
    for g in range(NSB):
        ne = int(Ktab[g].sum())
        for i in range(ne):
            colmap_edge[sb_e0[g] + i] = cc + i
        for j in range(SBW):
            colmap_self[g * SBW + j] = cc + ne + j
        cc += ne + SBW
    assert cc == S

    src_idx2 = np.zeros((M, 128, S), np.int32)
    tgt_idx2 = np.zeros((M, 128, S), np.int32)
    ee_idx2 = np.zeros((M, 128, S), np.int32)
    src_idx2[:, :, colmap_edge] = src_idx[:, :, :SE]
    tgt_idx2[:, :, colmap_edge] = tgt_idx[:, :, :SE]
    ee_idx2[:, :, colmap_edge] = ee_idx[:, :, :SE]
    lane = np.arange(128, dtype=np.int32)
    for c in range(M):
        for b in range(NB):
            rows = b * 128 + lane
            src_idx2[c, :, colmap_self[b]] = c * NPAD + rows
            tgt_idx2[c, :, colmap_self[b]] = rows
            ee_idx2[c, :, colmap_self[b]] = NPAIR + rows

    in_maps = []
    for c in range(M):
        im = dict(W)
        im["src_idx"] = src_idx2[c]
        im["tgt_idx"] = tgt_idx2[c]
        im["ee_idx"] = ee_idx2[c]
        im["trel"] = trel[c]
        im["x0row"] = x0row[c]
        im["bitsT"] = bitsT[c]
        im["brel"] = brel[c]
        im["pidx"] = pidx[c]
        im["lfT"] = lfT_h[c]
        im["rcg"] = rcg_h[c]
        in_maps.append(im)

    spec = {"Ktab": Ktab.tolist(), "SE": SE, "S": S}
    return in_maps, spec, pos_all


# ------------------------------------------------------------------ build
def build(spec, debug=False):
    Ktab = np.array(spec["Ktab"])
    SE, S = spec["SE"], spec["S"]
    ne_g = Ktab.sum(axis=1).astype(int)          # edge slots per sb
    ebase, sbase = [], []                        # col bases in [0,S)
    cc = 0
    for g in range(NSB):
        ebase.append(cc)
        sbase.append(cc + int(ne_g[g]))
        cc += int(ne_g[g]) + SBW
    assert cc == S
    tbase = np.concatenate([[0], np.cumsum(ne_g)]).astype(int)  # edge-only
    NEMAX = int(ne_g.max())
    NSLMAX = NEMAX + SBW

    nc = bacc.Bacc("TRN2", target_bir_lowering=False, debug=False,
                   enable_asserts=False, num_devices=M)

    def din(name, shape, dt=F32):
        return nc.dram_tensor(name, list(shape), dt, kind="ExternalInput").ap()

    t_srci = din("src_idx", [128, S], I32)
    t_tgti = din("tgt_idx", [128, S], I32)
    t_eei = din("ee_idx", [128, S], I32)
    t_trel = din("trel", [128, SE])
    t_x0row = din("x0row", [1, NPAD])
    t_bitsT = din("bitsT", [56, NPAD])
    t_brel = din("brel", [128, NB])
    t_pidx = din("pidx", [128, NB], I32)
    t_lfT = din("lfT", [128, NPAD], BF16)
    t_rcg = din("rcg", [128, GPC // 128])
    t_eetp = din("eetab_pairs", [L, NPAIR, 256])
    t_wcat = din("wcat", [L, 2, 128, 512], BF16)
    t_xlrb = din("xlr_b", [L, 128, 512])
    t_lew = din("lew", [L, 128, 256], BF16)
    t_att = din("att_rep", [L, 128, 256])
    t_cvb = din("convb_rep", [L, 128, 256])
    t_bng = din("bng", [L, 1, 256])
    t_bnb = din("bnb", [L, 1, 256])
    t_aemb = din("aemb_pad", [128, 128])
    t_alw = din("alw", [56, 128])
    t_albc = din("alb_col", [128, 1])
    t_iota = din("iota", [128, 128])
    t_mask = din("mask97", [128, 1])
    t_w1 = din("w1", [256, 1024], BF16)
    t_w2 = din("w2", [1024, 1024], BF16)
    t_w3 = din("w3", [1024, 512], BF16)
    t_w4 = din("w4", [512, NCLS], BF16)
    t_b1 = din("b1_rep", [128, 1024])
    t_b2 = din("b2_rep", [128, 1024])
    t_b3 = din("b3_rep", [128, 512])
    t_b4 = din("b4_rep", [128, NCLS])

    out_y = nc.dram_tensor("out_y", [GPC, NCLS], F32, kind="ExternalOutput").ap()
    dbg = {}
    if debug:
        for nm, shp in [("dbg_xl0", [NPAD, 256]), ("dbg_xr0", [NPAD, 256]),
                        ("dbg_see0", [NPAD, 256]),
                        ("dbg_out0", [NPAD, 256]), ("dbg_out1", [NPAD, 256]),
                        ("dbg_out2", [NPAD, 256]), ("dbg_out3", [NPAD, 256]),
                        ("dbg_pool", [BPAD, 256])]:
            dbg[nm] = nc.dram_tensor(nm, shp, F32, kind="ExternalOutput").ap()

    def sbrows(d, g, w=256):
        """DRAM rows for superblock g as [128, 7, w] AP matching SBUF wide."""
        return d[g * 896:(g + 1) * 896, :].rearrange("(b p) c -> p b c", p=128)

    with tile.TileContext(nc) as tc:
        with (
            tc.tile_pool(name="cst", bufs=1) as cst,
            tc.tile_pool(name="dram", bufs=1, space="DRAM") as dram,
        ):
            d_xl = dram.tile([NPAD, 256], F32)
            d_xr = dram.tile([NPAD, 256], F32)
            d_xl_alls = [dram.tile([M * NPAD, 256], F32, addr_space="Shared",
                                   name=f"xla{l}") for l in range(L)]
            d_eetabs = [dram.tile([NPAIR + NPAD, 256], F32, name=f"eet{l}")
                        for l in range(L)]
            d_st = dram.tile([128, SE * 128], F32)
            d_out = dram.tile([NPAD, 256], F32)
            d_pool = dram.tile([BPAD, 256], F32)
            d_pool_rs = dram.tile([GPC, 256], F32, name="poolrs")
            d_sin = dram.tile([1, 512], F32)
            d_souts = [dram.tile([1, 512], F32, addr_space="Shared",
                                 name=f"so{l}") for l in range(L)]

            for l in range(L):
                nc.sync.dma_start(d_eetabs[l][:NPAIR, :], t_eetp[l])

            # ---------------- persistent constants ----------------
            iota_f = cst.tile([128, 128], F32)
            nc.sync.dma_start(iota_f[:], t_iota[:])
            iotac = cst.tile([128, 1], F32)
            nc.sync.dma_start(iotac[:], t_iota[:].rearrange("a b -> b a")[:, :1])
            ones1 = cst.tile([1, 128], F32)
            nc.any.memset(ones1[:], 1.0)
            onesc = cst.tile([128, 1], F32)
            nc.any.memset(onesc[:], 1.0)
            mask97 = cst.tile([128, 1], F32)
            nc.sync.dma_start(mask97[:], t_mask[:])
            srci = cst.tile([128, S], I32)
            nc.sync.dma_start(srci[:], t_srci[:])
            tgti = cst.tile([128, S], I32)
            nc.sync.dma_start(tgti[:], t_tgti[:])
            eei = cst.tile([128, S], I32)
            nc.sync.dma_start(eei[:], t_eei[:])
            trelt = cst.tile([128, SE], F32)
            nc.sync.dma_start(trelt[:], t_trel[:])
            brelt = cst.tile([128, NB], F32)
            nc.sync.dma_start(brelt[:], t_brel[:])
            pidxt = cst.tile([128, NB], I32)
            nc.sync.dma_start(pidxt[:], t_pidx[:])
            lfT = cst.tile([128, NPAD], BF16)
            nc.sync.dma_start(lfT[:], t_lfT[:])
            rcgt = cst.tile([128, GPC // 128], F32)
            nc.sync.dma_start(rcgt[:], t_rcg[:])
            wcat_sb = []
            for l in range(L):
                row = []
                for h in range(2):
                    w = cst.tile([128, 512], BF16, name=f"wc{l}{h}")
                    nc.sync.dma_start(w[:], t_wcat[l, h])
                    row.append(w)
                wcat_sb.append(row)
            xlrb_sb = cst.tile([128, L * 512], F32)
            lew_sb = cst.tile([128, L * 256], BF16)
            att_sb = cst.tile([128, L * 256], F32)
            cvb_sb = cst.tile([128, L * 256], F32)
            for l in range(L):
                nc.sync.dma_start(xlrb_sb[:, l * 512:(l + 1) * 512], t_xlrb[l])
                nc.sync.dma_start(lew_sb[:, l * 256:(l + 1) * 256], t_lew[l])
                nc.sync.dma_start(att_sb[:, l * 256:(l + 1) * 256], t_att[l])
                nc.sync.dma_start(cvb_sb[:, l * 256:(l + 1) * 256], t_cvb[l])
            aemb_sb = cst.tile([128, 128], F32)
            nc.sync.dma_start(aemb_sb[:], t_aemb[:])
            alw_sb = cst.tile([56, 128], F32)
            nc.sync.dma_start(alw_sb[:], t_alw[:])
            albc = cst.tile([128, 1], F32)
            nc.sync.dma_start(albc[:], t_albc[:])

            # ---------------- phase A: st precompute ----------
            with tc.tile_pool(name="sbA", bufs=1) as sbA:
                for g in range(NSB):
                    ne = int(ne_g[g])
                    stw = sbA.tile([128, NEMAX * 128], F32, tag="stw")
                    for ss in range(ne):
                        tcol = tbase[g] + ss
                        nc.vector.tensor_scalar(
                            out=stw[:, ss * 128:(ss + 1) * 128], in0=iota_f[:],
                            scalar1=trelt[:, tcol:tcol + 1], scalar2=None,
                            op0=ALU.is_equal)
                    nc.sync.dma_start(
                        d_st[:, tbase[g] * 128:(tbase[g] + ne) * 128],
                        stw[:, :ne * 128])

            # ---------------- phase B: featurize -> layer-0 tables ----------
            with (
                tc.tile_pool(name="psB0", bufs=1, space="PSUM") as psB,
                tc.tile_pool(name="sbB0", bufs=1) as sbB,
            ):
                bitsT_sb = sbB.tile([56, NPAD], F32)
                nc.sync.dma_start(bitsT_sb[:], t_bitsT[:])
                x0_sb = sbB.tile([1, NPAD], F32)
                nc.sync.dma_start(x0_sb[:], t_x0row[:])
                gblocks = [(i * 4, min(4, NB - i * 4)) for i in range((NB + 3) // 4)]
                for (b0, nb) in gblocks:
                    nn_ = nb * 128
                    rep_ps = psB.tile([128, 512], F32, space="PSUM", tag="rep")
                    nc.tensor.matmul(rep_ps[:, :nn_], lhsT=ones1[:],
                                     rhs=x0_sb[:, b0 * 128:b0 * 128 + nn_],
                                     start=True, stop=True)
                    oh = sbB.tile([128, 512], F32, tag="oh")
                    nc.vector.tensor_scalar(out=oh[:, :nn_], in0=rep_ps[:, :nn_],
                                            scalar1=iotac[:, :1], scalar2=None,
                                            op0=ALU.is_equal)
                    top_ps = psB.tile([128, 512], F32, space="PSUM", tag="top")
                    nc.tensor.matmul(top_ps[:, :nn_], lhsT=aemb_sb[:],
                                     rhs=oh[:, :nn_], start=True, stop=True)
                    bot_ps = psB.tile([128, 512], F32, space="PSUM", tag="bot")
                    nc.tensor.matmul(bot_ps[:, :nn_], lhsT=alw_sb[:],
                                     rhs=bitsT_sb[:, b0 * 128:b0 * 128 + nn_],
                                     start=True, stop=True)
                    topb = sbB.tile([128, 512], BF16, tag="topb")
                    nc.vector.tensor_scalar(out=topb[:, :nn_], in0=top_ps[:, :nn_],
                                            scalar1=1.0, scalar2=None,
                                            op0=ALU.mult)
                    botb = sbB.tile([128, 512], BF16, tag="botb")
                    nc.vector.tensor_scalar(out=botb[:, :nn_], in0=bot_ps[:, :nn_],
                                            scalar1=albc[:, :1], scalar2=None,
                                            op0=ALU.add)
                    xlrw = sbB.tile([128, 4 * 512], F32, tag="xlrw")
                    seew = sbB.tile([128, 4 * 256], F32, tag="seew")
                    for i in range(nb):
                        b = b0 + i
                        xlr_ps = psB.tile([128, 512], F32, space="PSUM",
                                          tag="xlr")
                        nc.tensor.matmul(xlr_ps[:],
                                         lhsT=topb[:, i * 128:(i + 1) * 128],
                                         rhs=wcat_sb[0][0][:], start=True,
                                         stop=False)
                        nc.tensor.matmul(xlr_ps[:],
                                         lhsT=botb[:, i * 128:(i + 1) * 128],
                                         rhs=wcat_sb[0][1][:], start=False,
                                         stop=True)
                        see_ps = psB.tile([128, 256], F32, space="PSUM",
                                          tag="see")
                        nc.tensor.matmul(see_ps[:],
                                         lhsT=lfT[:, b * 128:(b + 1) * 128],
                                         rhs=lew_sb[:, :256], start=True,
                                         stop=True)
                        nc.vector.tensor_tensor(
                            out=xlrw[:, i * 512:(i + 1) * 512], in0=xlr_ps[:],
                            in1=xlrb_sb[:, :512], op=ALU.add)
                        nc.vector.tensor_copy(seew[:, i * 256:(i + 1) * 256],
                                              see_ps[:])
                    rows = slice(b0 * 128, b0 * 128 + nn_)
                    nc.sync.dma_start(
                        d_xl[rows, :].rearrange("(b p) c -> p b c", p=128),
                        xlrw[:, :nb * 512].rearrange(
                            "p (b c) -> p b c", b=nb)[:, :, 0:256])
                    nc.sync.dma_start(
                        d_xr[rows, :].rearrange("(b p) c -> p b c", p=128),
                        xlrw[:, :nb * 512].rearrange(
                            "p (b c) -> p b c", b=nb)[:, :, 256:512])
                    nc.sync.dma_start(
                        d_eetabs[0][NPAIR + b0 * 128:NPAIR + b0 * 128 + nn_, :]
                        .rearrange("(b p) c -> p b c", p=128),
                        seew[:, :nb * 256].rearrange("p (b c) -> p b c", b=nb))

            if debug:
                nc.sync.dma_start(dbg["dbg_xl0"][:], d_xl.opt())
                nc.sync.dma_start(dbg["dbg_xr0"][:], d_xr.opt())
                nc.sync.dma_start(dbg["dbg_see0"][:],
                                  d_eetabs[0][NPAIR:, :])

            # ---------------- conv layers ----------------
            for l in range(L):
                Wyp = 256 + (HEADS if l == 0 else 1)
                H = HEADS if l == 0 else 1
                CD = 256 // H
                lsl = slice(l * 256, (l + 1) * 256)
                nc.gpsimd.collective_compute(
                    "AllGather", ALU.bypass, ins=[d_xl.opt()],
                    outs=[d_xl_alls[l].opt()], replica_groups=[list(range(M))])

                with (
                    tc.tile_pool(name=f"psE{l}", bufs=1, space="PSUM") as psE,
                    tc.tile_pool(name=f"sbE{l}", bufs=1) as sbE,
                ):
                    stats_ps = psE.tile([1, 512], F32, space="PSUM", tag="stats")
                    for g in range(NSB):
                        ne = int(ne_g[g])
                        nsl = ne + SBW
                        xls = sbE.tile([128, NSLMAX * 256], F32, tag="xls")
                        xrg = sbE.tile([128, NSLMAX * 256], F32, tag="xrg")
                        v = sbE.tile([128, NSLMAX * 256], F32, tag="v")
                        for s in range(ne):
                            col = ebase[g] + s
                            nc.gpsimd.indirect_dma_start(
                                out=xls[:, s * 256:(s + 1) * 256],
                                out_offset=None, in_=d_xl_alls[l].opt(),
                                in_offset=bass.IndirectOffsetOnAxis(
                                    ap=srci[:, col:col + 1], axis=0))
                            nc.gpsimd.indirect_dma_start(
                                out=xrg[:, s * 256:(s + 1) * 256],
                                out_offset=None, in_=d_xr.opt(),
                                in_offset=bass.IndirectOffsetOnAxis(
                                    ap=tgti[:, col:col + 1], axis=0))
                            nc.gpsimd.indirect_dma_start(
                                out=v[:, s * 256:(s + 1) * 256],
                                out_offset=None, in_=d_eetabs[l].opt(),
                                in_offset=bass.IndirectOffsetOnAxis(
                                    ap=eei[:, col:col + 1], axis=0))
                        selfsl = slice(ne * 256, nsl * 256)
                        nc.sync.dma_start(
                            xls[:, selfsl].rearrange("p (b c) -> p b c", b=SBW),
                            sbrows(d_xl, g))
                        nc.sync.dma_start(
                            xrg[:, selfsl].rearrange("p (b c) -> p b c", b=SBW),
                            sbrows(d_xr, g))
                        nc.sync.dma_start(
                            v[:, selfsl].rearrange("p (b c) -> p b c", b=SBW),
                            d_eetabs[l][NPAIR + g * 896:NPAIR + (g + 1) * 896, :]
                            .rearrange("(b p) c -> p b c", p=128))
                        wv = slice(0, nsl * 256)
                        nc.vector.tensor_tensor(out=v[:, wv], in0=v[:, wv],
                                                in1=xls[:, wv], op=ALU.add)
                        nc.vector.tensor_tensor(out=v[:, wv], in0=v[:, wv],
                                                in1=xrg[:, wv], op=ALU.add)
                        nc.vector.scalar_tensor_tensor(
                            out=v[:, wv], in0=v[:, wv], scalar=0.2,
                            in1=v[:, wv], op0=ALU.mult, op1=ALU.max)
                        am = sbE.tile([128, NSLMAX * 256], F32, tag="xrg")
                        nc.vector.tensor_tensor(
                            out=am[:, wv].rearrange("p (s c) -> p s c", s=nsl),
                            in0=v[:, wv].rearrange("p (s c) -> p s c", s=nsl),
                            in1=att_sb[:, lsl].rearrange("p (u c) -> p u c", u=1)
                                .broadcast_to([128, nsl, 256]), op=ALU.mult)
                        ypw = sbE.tile([128, NSLMAX * 264], F32, tag="ypw")
                        yv = ypw[:, :nsl * Wyp].rearrange("p (s w) -> p s w",
                                                          w=Wyp)
                        nc.vector.reduce_sum(
                            yv[:, :, 256:Wyp],
                            am[:, wv].rearrange("p (s h c) -> p s h c",
                                                s=nsl, h=H), axis=AX)
                        nc.scalar.activation(yv[:, :, 256:Wyp],
                                             yv[:, :, 256:Wyp], ACTF.Exp)
                        nc.vector.tensor_tensor(
                            out=yv[:, :, 0:256].rearrange(
                                "p s (h c) -> p s h c", h=H),
                            in0=xls[:, wv].rearrange("p (s h c) -> p s h c",
                                                     s=nsl, h=H),
                            in1=yv[:, :, 256:Wyp].rearrange(
                                "p s (h u) -> p s h u", u=1)
                                .broadcast_to([128, nsl, H, CD]), op=ALU.mult)
                        stw = sbE.tile([128, NEMAX * 128], F32, tag="stw")
                        if ne > 0:
                            nc.sync.dma_start(
                                stw[:, :ne * 128],
                                d_st[:, tbase[g] * 128:(tbase[g] + ne) * 128])
                        ndps = [psE.tile([128, Wyp], F32, space="PSUM",
                                         tag=f"nd{j}", name=f"ndps{j}")
                                for j in range(SBW)]
                        s = 0
                        for j in range(SBW):
                            K = int(Ktab[g, j])
                            for k in range(K):
                                nc.tensor.matmul(
                                    ndps[j][:],
                                    lhsT=stw[:, s * 128:(s + 1) * 128],
                                    rhs=ypw[:, s * Wyp:(s + 1) * Wyp],
                                    start=(k == 0), stop=(k == K - 1))
                                s += 1
                        ndw = sbE.tile([128, SBW * 264], F32, tag="ndw")
                        for j in range(SBW):
                            ssl = slice((ne + j) * Wyp, (ne + j + 1) * Wyp)
                            osl = slice(j * Wyp, (j + 1) * Wyp)
                            if int(Ktab[g, j]) == 0:
                                nc.vector.tensor_copy(ndw[:, osl], ypw[:, ssl])
                            else:
                                nc.vector.tensor_tensor(
                                    out=ndw[:, osl], in0=ndps[j][:],
                                    in1=ypw[:, ssl], op=ALU.add)
                        nv = ndw[:, :SBW * Wyp].rearrange("p (s w) -> p s w",
                                                          w=Wyp)
                        rdn = sbE.tile([128, SBW * 8], F32, tag="rdn")
                        nc.vector.reciprocal(rdn[:, :SBW * H],
                                             nv[:, :, 256:Wyp])
                        sqi = sbE.tile([128, SBW * 512], F32, tag="sqi")
                        sv = sqi[:].rearrange("p (s w) -> p s w", w=512)
                        xv = sv[:, :, 0:256]
                        nc.vector.tensor_tensor(
                            out=xv.rearrange("p s (h c) -> p s h c", h=H),
                            in0=nv[:, :, 0:256].rearrange(
                                "p s (h c) -> p s h c", h=H),
                            in1=rdn[:, :SBW * H].rearrange(
                                "p (s h u) -> p s h u", s=SBW, u=1)
                                .broadcast_to([128, SBW, H, CD]), op=ALU.mult)
                        nc.vector.tensor_tensor(
                            out=xv, in0=xv,
                            in1=cvb_sb[:, lsl].rearrange("p (u c) -> p u c", u=1)
                                .broadcast_to([128, SBW, 256]), op=ALU.add)
                        if g == NSB - 1:
                            nc.vector.tensor_scalar(
                                out=sqi[:, 6 * 512:6 * 512 + 256],
                                in0=sqi[:, 6 * 512:6 * 512 + 256],
                                scalar1=mask97[:, :1], scalar2=None,
                                op0=ALU.mult)
                        nc.vector.tensor_tensor(
                            out=sv[:, :, 256:512], in0=xv, in1=xv,
                            op=ALU.mult)
                        for j in range(SBW):
                            nc.tensor.matmul(
                                stats_ps[:], lhsT=onesc[:],
                                rhs=sqi[:, j * 512:(j + 1) * 512],
                                start=(g == 0 and j == 0),
                                stop=(g == NSB - 1 and j == SBW - 1))
                        nc.sync.dma_start(sbrows(d_out, g),
                                          sv[:, :, 0:256])
                    st_sb = sbE.tile([1, 512], F32, tag="stsb")
                    nc.vector.tensor_copy(st_sb[:], stats_ps[:])
                    nc.sync.dma_start(d_sin[:], st_sb[:])

                nc.gpsimd.collective_compute(
                    "AllReduce", ALU.add, ins=[d_sin.opt()],
                    outs=[d_souts[l].opt()], replica_groups=[list(range(M))])
                if debug:
                    nc.sync.dma_start(dbg[f"dbg_out{l}"][:], d_out.opt())

                # ---- BN coeffs ----
                with (
                    tc.tile_pool(name=f"psP{l}", bufs=1, space="PSUM") as psP,
                    tc.tile_pool(name=f"sbP{l}", bufs=1) as sbP,
                ):
                    stg = sbP.tile([1, 512], F32)
                    nc.sync.dma_start(stg[:], d_souts[l].opt())
                    rowAB = sbP.tile([1, 512], F32)
                    mu = sbP.tile([1, 256], F32)
                    nc.vector.tensor_scalar(out=mu[:], in0=stg[:, :256],
                                            scalar1=1.0 / N, scalar2=None,
                                            op0=ALU.mult)
                    ex2 = sbP.tile([1, 256], F32)
                    nc.vector.tensor_scalar(out=ex2[:], in0=stg[:, 256:],
                                            scalar1=1.0 / N, scalar2=None,
                                            op0=ALU.mult)
                    var = sbP.tile([1, 256], F32)
                    nc.vector.tensor_tensor(out=var[:], in0=mu[:], in1=mu[:],
                                            op=ALU.mult)
                    nc.vector.tensor_tensor(out=var[:], in0=ex2[:], in1=var[:],
                                            op=ALU.subtract)
                    nc.vector.tensor_scalar(out=var[:], in0=var[:],
                                            scalar1=1e-5, scalar2=None,
                                            op0=ALU.add)
                    sd = sbP.tile([1, 256], F32)
                    nc.scalar.activation(sd[:], var[:], ACTF.Sqrt)
                    rstd = sbP.tile([1, 256], F32)
                    nc.vector.reciprocal(rstd[:], sd[:])
                    bng = sbP.tile([1, 256], F32)
                    nc.sync.dma_start(bng[:], t_bng[l])
                    bnb = sbP.tile([1, 256], F32)
                    nc.sync.dma_start(bnb[:], t_bnb[l])
                    nc.vector.tensor_tensor(out=rowAB[:, :256], in0=rstd[:],
                                            in1=bng[:], op=ALU.mult)
                    t3 = sbP.tile([1, 256], F32)
                    nc.vector.tensor_tensor(out=t3[:], in0=mu[:],
                                            in1=rowAB[:, :256], op=ALU.mult)
                    nc.vector.tensor_tensor(out=rowAB[:, 256:], in0=bnb[:],
                                            in1=t3[:], op=ALU.subtract)
                    rab_ps = psP.tile([128, 512], F32, space="PSUM", tag="rab")
                    nc.tensor.matmul(rab_ps[:], lhsT=ones1[:], rhs=rowAB[:],
                                     start=True, stop=True)
                    rab = sbP.tile([128, 512], F32)
                    nc.vector.tensor_copy(rab[:], rab_ps[:])

                    if l < L - 1:
                        # ---- pass B: next-layer tables ----
                        for g in range(NSB):
                            xnw = sbP.tile([128, SBW * 256], F32, tag="xnw")
                            nc.sync.dma_start(
                                xnw[:].rearrange("p (b c) -> p b c", b=SBW),
                                sbrows(d_out, g))
                            nc.vector.tensor_tensor(
                                out=xnw[:].rearrange("p (s c) -> p s c", s=SBW),
                                in0=xnw[:].rearrange("p (s c) -> p s c", s=SBW),
                                in1=rab[:, :256].rearrange("p (u c) -> p u c",
                                                           u=1)
                                    .broadcast_to([128, SBW, 256]), op=ALU.mult)
                            nc.vector.tensor_tensor(
                                out=xnw[:].rearrange("p (s c) -> p s c", s=SBW),
                                in0=xnw[:].rearrange("p (s c) -> p s c", s=SBW),
                                in1=rab[:, 256:].rearrange("p (u c) -> p u c",
                                                           u=1)
                                    .broadcast_to([128, SBW, 256]), op=ALU.add)
                            xnb = sbP.tile([128, SBW * 256], BF16, tag="xnb")
                            nc.vector.scalar_tensor_tensor(
                                out=xnb[:], in0=xnw[:], scalar=0.01,
                                in1=xnw[:], op0=ALU.mult, op1=ALU.max)
                            xlrw = sbP.tile([128, SBW * 512], F32, tag="xlrw")
                            seew = sbP.tile([128, SBW * 256], F32, tag="seew")
                            for j in range(SBW):
                                b = g * SBW + j
                                xnT = sbP.tile([128, 256], BF16, tag="xnT")
                                for h in range(2):
                                    nc.sync.dma_start_transpose(
                                        xnT[:, h * 128:(h + 1) * 128],
                                        xnb[:, j * 256 + h * 128:
                                            j * 256 + (h + 1) * 128])
                                xlr_ps = psP.tile([128, 512], F32,
                                                  space="PSUM", tag="xlr")
                                for h in range(2):
                                    nc.tensor.matmul(
                                        xlr_ps[:],
                                        lhsT=xnT[:, h * 128:(h + 1) * 128],
                                        rhs=wcat_sb[l + 1][h][:],
                                        start=(h == 0), stop=(h == 1))
                                see_ps = psP.tile([128, 256], F32,
                                                  space="PSUM", tag="see")
                                nc.tensor.matmul(
                                    see_ps[:],
                                    lhsT=lfT[:, b * 128:(b + 1) * 128],
                                    rhs=lew_sb[:, (l + 1) * 256:(l + 2) * 256],
                                    start=True, stop=True)
                                nc.vector.tensor_tensor(
                                    out=xlrw[:, j * 512:(j + 1) * 512],
                                    in0=xlr_ps[:],
                                    in1=xlrb_sb[:, (l + 1) * 512:(l + 2) * 512],
                                    op=ALU.add)
                                nc.vector.tensor_copy(
                                    seew[:, j * 256:(j + 1) * 256], see_ps[:])
                            nc.sync.dma_start(
                                sbrows(d_xl, g),
                                xlrw[:].rearrange("p (b c) -> p b c",
                                                  b=SBW)[:, :, 0:256])
                            nc.sync.dma_start(
                                sbrows(d_xr, g),
                                xlrw[:].rearrange("p (b c) -> p b c",
                                                  b=SBW)[:, :, 256:512])
                            nc.sync.dma_start(
                                d_eetabs[l + 1][NPAIR + g * 896:
                                                NPAIR + (g + 1) * 896, :]
                                .rearrange("(b p) c -> p b c", p=128),
                                seew[:].rearrange("p (b c) -> p b c", b=SBW))
                    else:
                        # ---- pooling ----
                        zer = sbP.tile([128, 256], F32, tag="zer")
                        nc.any.memset(zer[:], 0.0)
                        for i in range(BPAD // 128):
                            nc.sync.dma_start(d_pool[i * 128:(i + 1) * 128, :],
                                              zer[:])
                        for g in range(NSB):
                            xnw = sbP.tile([128, SBW * 256], F32, tag="xnw")
                            nc.sync.dma_start(
                                xnw[:].rearrange("p (b c) -> p b c", b=SBW),
                                sbrows(d_out, g))
                            nc.vector.tensor_tensor(
                                out=xnw[:].rearrange("p (s c) -> p s c", s=SBW),
                                in0=xnw[:].rearrange("p (s c) -> p s c", s=SBW),
                                in1=rab[:, :256].rearrange("p (u c) -> p u c",
                                                           u=1)
                                    .broadcast_to([128, SBW, 256]), op=ALU.mult)
                            nc.vector.tensor_tensor(
                                out=xnw[:].rearrange("p (s c) -> p s c", s=SBW),
                                in0=xnw[:].rearrange("p (s c) -> p s c", s=SBW),
                                in1=rab[:, 256:].rearrange("p (u c) -> p u c",
                                                           u=1)
                                    .broadcast_to([128, SBW, 256]), op=ALU.add)
                            for j in range(SBW):
                                b = g * SBW + j
                                ohp = sbP.tile([128, 128], F32, tag="ohp")
                                nc.vector.tensor_scalar(
                                    out=ohp[:], in0=iota_f[:],
                                    scalar1=brelt[:, b:b + 1], scalar2=None,
                                    op0=ALU.is_equal)
                                pool_ps = psP.tile([128, 256], F32,
                                                   space="PSUM", tag="pool")
                                nc.tensor.matmul(
                                    pool_ps[:], lhsT=ohp[:],
                                    rhs=xnw[:, j * 256:(j + 1) * 256],
                                    start=True, stop=True)
                                pool_sb = sbP.tile([128, 256], F32, tag="poolsb")
                                nc.vector.tensor_copy(pool_sb[:], pool_ps[:])
                                nc.gpsimd.indirect_dma_start(
                                    out=d_pool.opt(), in_=pool_sb[:],
                                    in_offset=None,
                                    out_offset=bass.IndirectOffsetOnAxis(
                                        ap=pidxt[:, b:b + 1], axis=0),
                                    compute_op=ALU.add)

            if debug:
                nc.sync.dma_start(dbg["dbg_pool"][:], d_pool.opt())
            nc.gpsimd.collective_compute(
                "ReduceScatter", ALU.add, ins=[d_pool[:B, :]],
                outs=[d_pool_rs.opt()], replica_groups=[list(range(M))])

            # ---------------- MLP ----------------
            with (
                tc.tile_pool(name="psM", bufs=1, space="PSUM") as psM,
                tc.tile_pool(name="sbM", bufs=1) as sbM,
                tc.tile_pool(name="wM", bufs=1) as wM,
            ):
                w1s = wM.tile([128, 2 * 1024], BF16)
                for i in range(2):
                    nc.sync.dma_start(w1s[:, i * 1024:(i + 1) * 1024],
                                      t_w1[i * 128:(i + 1) * 128, :])
                w2s = wM.tile([128, 8 * 1024], BF16)
                for i in range(8):
                    nc.sync.dma_start(w2s[:, i * 1024:(i + 1) * 1024],
                                      t_w2[i * 128:(i + 1) * 128, :])
                w3s = wM.tile([128, 8 * 512], BF16)
                for i in range(8):
                    nc.sync.dma_start(w3s[:, i * 512:(i + 1) * 512],
                                      t_w3[i * 128:(i + 1) * 128, :])
                w4s = wM.tile([128, 4 * NCLS], BF16)
                for i in range(4):
                    nc.sync.dma_start(w4s[:, i * NCLS:(i + 1) * NCLS],
                                      t_w4[i * 128:(i + 1) * 128, :])
                b1s = wM.tile([128, 1024], F32)
                nc.sync.dma_start(b1s[:], t_b1[:])
                b2s = wM.tile([128, 1024], F32)
                nc.sync.dma_start(b2s[:], t_b2[:])
                b3s = wM.tile([128, 512], F32)
                nc.sync.dma_start(b3s[:], t_b3[:])
                b4s = wM.tile([128, NCLS], F32)
                nc.sync.dma_start(b4s[:], t_b4[:])

                def ffn(xT, xwidth, ws, wwidth, bs, tagp):
                    """y = relu(x @ W + b) given xT bf16 [128, xwidth] (slices
                    of 128), ws [128, (xwidth/128)*wwidth]; returns f32 tile."""
                    nin = xwidth // 128
                    nps = (wwidth + 511) // 512
                    hf = sbM.tile([128, wwidth], F32, tag=f"hf{tagp}")
                    for np_ in range(nps):
                        wlo = np_ * 512
                        whi = min(wwidth, wlo + 512)
                        hp = psM.tile([128, 512], F32, space="PSUM",
                                      tag=f"hp{np_}")
                        for kk in range(nin):
                            nc.tensor.matmul(
                                hp[:, :whi - wlo],
                                lhsT=xT[:, kk * 128:(kk + 1) * 128],
                                rhs=ws[:, kk * wwidth + wlo:kk * wwidth + whi],
                                start=(kk == 0), stop=(kk == nin - 1))
                        nc.vector.tensor_tensor(out=hf[:, wlo:whi],
                                                in0=hp[:, :whi - wlo],
                                                in1=bs[:, wlo:whi], op=ALU.add)
                    return hf

                def transp(hf, width, dorelu=True):
                    hb = sbM.tile([128, width], BF16, tag=f"hb{width}")
                    if dorelu:
                        nc.scalar.activation(hb[:], hf[:], ACTF.Relu)
                    else:
                        nc.vector.tensor_copy(hb[:], hf[:])
                    hT = sbM.tile([128, width], BF16, tag=f"hT{width}")
                    for i in range(width // 128):
                        nc.sync.dma_start_transpose(
                            hT[:, i * 128:(i + 1) * 128],
                            hb[:, i * 128:(i + 1) * 128])
                    return hT

                for i in range(GPC // 128):
                    pc = sbM.tile([128, 256], F32, tag="pc")
                    nc.sync.dma_start(pc[:], d_pool_rs[i * 128:(i + 1) * 128, :])
                    g0 = sbM.tile([128, 256], F32, tag="g0")
                    nc.vector.tensor_scalar(out=g0[:], in0=pc[:],
                                            scalar1=rcgt[:, i:i + 1],
                                            scalar2=None, op0=ALU.mult)
                    gT = transp(g0, 256, dorelu=False)
                    h1 = ffn(gT, 256, w1s, 1024, b1s, "1")
                    h1T = transp(h1, 1024)
                    h2 = ffn(h1T, 1024, w2s, 1024, b2s, "2")
                    h2T = transp(h2, 1024)
                    h3 = ffn(h2T, 1024, w3s, 512, b3s, "3")
                    h3T = transp(h3, 512)
                    yp = psM.tile([128, NCLS], F32, space="PSUM", tag="yp")
                    for kk in range(4):
                        nc.tensor.matmul(yp[:],
                                         lhsT=h3T[:, kk * 128:(kk + 1) * 128],
                                         rhs=w4s[:, kk * NCLS:(kk + 1) * NCLS],
                                         start=(kk == 0), stop=(kk == 3))
                    yo = sbM.tile([128, NCLS], F32, tag="yo")
                    nc.vector.tensor_tensor(out=yo[:], in0=yp[:], in1=b4s[:],
                                            op=ALU.add)
                    nc.sync.dma_start(out_y[i * 128:(i + 1) * 128, :], yo[:])

    nc.compile()
    return nc


# ------------------------------------------------------------------ entry
def kernel(**inputs) -> np.ndarray:
    in_maps, spec, _ = host_prep(inputs)
    key = (spec["SE"], str(spec["Ktab"]))
    if key not in _cache:
        _cache[key] = build(spec)
    nc = _cache[key]
    res = run_bass_kernel_spmd(nc, in_maps, list(range(M)))
    return np.concatenate([res.results[c]["out_y"] for c in range(M)], axis=0)



# revision 3
# speedup vs baseline: 4.5192x; 4.5192x over previous
"""TRN2 Bass kernel for nn_GAT_73950746902569 — hardware-loop v3.

The runtime charges ~42us per STATIC instruction (nearly independent of data
size and of dynamic trip counts), so v3 wraps every repeated structure in a
For_i hardware loop with a uniform body: uniform slot counts per superblock
(Kvec caps packing), index/one-hot tables staged from DRAM per iteration with
dynamic slices, and all SBUF addressing static inside loop bodies.
Static instruction count ~1.2k vs ~14k for the unrolled v2.
"""
import numpy as np
import ml_dtypes

import concourse.bass as bass
import concourse.bacc as bacc
import concourse.mybir as mybir
import concourse.tile as tile
from concourse.bass import ds, ts
from concourse.bass_utils import run_bass_kernel_spmd

N, E, B = 100000, 200000, 4096
HID, EDIM, HEADS, L, NCLS = 256, 64, 8, 4, 3
M = 8
NPC = N // M            # 12500
NB = 98
NPAD = NB * 128         # 12544
NB2 = 100               # featurize block padding (25 groups of 4)
NPAD2 = NB2 * 128       # 12800
SBW = 7                 # blocks per superblock
NSB = NB // SBW         # 14
GPC = B // M            # 512
BPAD = 4224
NPAIR = 484             # 22*22
P = 128

F32 = mybir.dt.float32
BF16 = mybir.dt.bfloat16
I32 = mybir.dt.int32
ALU = mybir.AluOpType
ACTF = mybir.ActivationFunctionType
AX = mybir.AxisListType.X

_cache = {}

CONFIGS = [
    [3, 2, 2, 2, 2, 2, 2],
    [3, 3, 2, 2, 2, 2, 2],
    [3, 3, 3, 2, 2, 2, 2],
    [3, 3, 3, 3, 2, 2, 2],
    [3, 3, 3, 3, 3, 2, 2],
    [3, 3, 3, 3, 3, 3, 3],
]


def _bits(a):
    """[n] uint -> [n,8] f32 bits MSB-first."""
    return (((np.asarray(a)[:, None] >> np.arange(7, -1, -1)) & 1)
            .astype(np.float32))


def _bits_rows(a):
    """[n,k] -> [n,8k] f32 MSB-first per byte."""
    a = np.asarray(a)
    bits = ((a[:, :, None] >> np.arange(7, -1, -1)) & 1)
    return bits.reshape(a.shape[0], -1).astype(np.float32)


def _rep(v, n=128):
    v = np.asarray(v, np.float32)
    return np.broadcast_to(v[None, :], (n, v.shape[-1])).copy()


def _pack_sb(deg, caps_e, caps_n):
    """FFD nodes (deg desc) into 7 blocks with edge+node caps.
    Returns (block, lane) per node or None if infeasible."""
    order = np.argsort(-deg, kind="stable")
    ne = np.zeros(SBW, np.int64)
    nn_ = np.zeros(SBW, np.int64)
    blk = np.empty(len(deg), np.int64)
    lane = np.empty(len(deg), np.int64)
    for i in order:
        di = deg[i]
        for j in range(SBW):
            if nn_[j] < caps_n[j] and ne[j] + di <= caps_e[j]:
                blk[i] = j
                lane[i] = nn_[j]
                nn_[j] += 1
                ne[j] += di
                break
        else:
            return None
    return blk, lane, nn_


def host_prep(inputs):
    x = np.asarray(inputs["x"])
    edge_index = np.asarray(inputs["edge_index"])
    edge_attr = np.asarray(inputs["edge_attr"])
    batch = np.asarray(inputs["batch"])

    src, tgt = edge_index[0].astype(np.int64), edge_index[1].astype(np.int64)
    pair = (edge_attr[:, 0] * 22 + edge_attr[:, 1]).astype(np.int64)

    # ---- weight-derived tables (shared across cores) ----
    atom_emb = np.asarray(inputs["atom_emb"], np.float32)        # [120,128]
    alw = np.asarray(inputs["atom_lin_w"], np.float32)           # [56,128]
    alb = np.asarray(inputs["atom_lin_b"], np.float32)           # [128]
    edge_emb = np.asarray(inputs["edge_emb"], np.float32)        # [22,64]
    elw = np.asarray(inputs["edge_lin_w"], np.float32)           # [8,64]
    elb = np.asarray(inputs["edge_lin_b"], np.float32)           # [64]
    lin_l_w = np.asarray(inputs["lin_l_w"], np.float32)
    lin_r_w = np.asarray(inputs["lin_r_w"], np.float32)
    lin_e_w = np.asarray(inputs["lin_e_w"], np.float32)

    a0g, a1g = np.meshgrid(np.arange(22), np.arange(22), indexing="ij")
    ef_pairs = np.concatenate(
        [edge_emb[a0g.ravel()], _bits(a1g.ravel()) @ elw + elb],
        axis=1).astype(np.float32)                               # [484,128]
    eetab_pairs = np.stack(
        [ef_pairs @ lin_e_w[l] for l in range(L)]).astype(np.float32)

    W = {}
    W["eetab_pairs"] = eetab_pairs                              # [L,484,256]
    W["wcat"] = np.stack([
        np.stack([np.concatenate([lin_l_w[l, 128 * h:128 * (h + 1)],
                                  lin_r_w[l, 128 * h:128 * (h + 1)]], axis=1)
                  for h in range(2)]) for l in range(L)
    ]).astype(ml_dtypes.bfloat16)                               # [L,2,128,512]
    W["xlr_b"] = np.stack([
        _rep(np.concatenate([np.asarray(inputs["lin_l_b"])[l],
                             np.asarray(inputs["lin_r_b"])[l]]))
        for l in range(L)])                                     # [L,128,512]
    W["lew"] = lin_e_w.astype(ml_dtypes.bfloat16)               # [L,128,256]
    W["att_rep"] = np.stack([_rep(np.asarray(inputs["att"])[l])
                             for l in range(L)])
    W["convb_rep"] = np.stack([_rep(np.asarray(inputs["conv_b"])[l])
                               for l in range(L)])
    W["bng"] = np.asarray(inputs["bn_g"], np.float32)[:, None, :]
    W["bnb"] = np.asarray(inputs["bn_b"], np.float32)[:, None, :]
    aemb_pad = np.zeros((128, 128), np.float32)
    aemb_pad[:120] = atom_emb
    W["aemb_pad"] = aemb_pad
    W["alw"] = alw
    W["alb_col"] = alb[:, None].astype(np.float32)              # [128,1]
    W["iota"] = np.broadcast_to(np.arange(128, dtype=np.float32)[None, :],
                                (128, 128)).copy()
    for k in ("w1", "w2", "w3", "w4"):
        W[k] = np.asarray(inputs[k], np.float32).astype(ml_dtypes.bfloat16)
    for k in ("b1", "b2", "b3", "b4"):
        W[k + "_rep"] = _rep(np.asarray(inputs[k]))

    # ---- loop_attr (input-derived) ----
    deg_all = np.bincount(tgt, minlength=N)
    order = np.argsort(tgt, kind="stable")
    ef_e = ef_pairs[pair[order]]                                # [E,128] f32
    starts = np.searchsorted(tgt[order], np.arange(N + 1))
    nonempty = deg_all > 0
    la = np.zeros((N, 128), np.float32)
    la[nonempty] = np.add.reduceat(ef_e, starts[:-1][nonempty], axis=0)
    la /= np.maximum(deg_all, 1)[:, None]
    gcnt = np.bincount(np.asarray(batch, np.int64), minlength=B)
    rcg_all = (1.0 / np.maximum(gcnt, 1)).astype(np.float32)

    # ---- per-core packing: uniform Kvec caps across all cores/superblocks --
    for kv in CONFIGS:
        caps_e = [k * 128 for k in kv]
        packs = []
        ok = True
        for c in range(M):
            dd = deg_all[c * NPC:(c + 1) * NPC]
            core_packs = []
            for g in range(NSB):
                lo, hi = g * 896, min((g + 1) * 896, NPC)
                caps_n = [128] * SBW
                if hi - lo < 896:
                    caps_n[SBW - 1] = hi - lo - 128 * (SBW - 1)
                r = _pack_sb(dd[lo:hi], caps_e, caps_n)
                if r is None:
                    ok = False
                    break
                core_packs.append(r)
            if not ok:
                break
            packs.append(core_packs)
        if ok:
            Kvec = kv
            break
    else:
        raise RuntimeError("no feasible packing config")
    NSLOT = int(sum(Kvec))
    sbase = np.concatenate([[0], np.cumsum(Kvec)]).astype(int)

    pos_all = np.empty(N, np.int64)
    nfill = np.zeros((M, NB), np.int64)
    for c in range(M):
        for g in range(NSB):
            lo, hi = g * 896, min((g + 1) * 896, NPC)
            blk, lane, nn_ = packs[c][g]
            pos_all[c * NPC + lo:c * NPC + hi] = \
                (g * SBW + blk) * 128 + lane
            nfill[c, g * SBW:(g + 1) * SBW] = nn_
    gpad = (np.arange(N) // NPC) * NPAD + pos_all

    lfT_h = np.zeros((M, 128, NPAD2), ml_dtypes.bfloat16)
    rcg_h = np.zeros((M, 128, GPC // 128), np.float32)
    for c in range(M):
        sl = slice(c * NPC, (c + 1) * NPC)
        laT = np.zeros((128, NPAD2), np.float32)
        laT[:, pos_all[sl]] = la[sl].T
        lfT_h[c] = laT.astype(ml_dtypes.bfloat16)
        rcg_h[c] = rcg_all[c * GPC:(c + 1) * GPC].reshape(
            GPC // 128, 128).T

    idx3 = np.zeros((M, 128, NSB * 3 * NSLOT), np.int32)
    trel = np.full((M, 128, NSB * NSLOT), 200.0, np.float32)
    mask7 = np.zeros((M, 128, NB), np.float32)
    x0row = np.zeros((M, 1, NPAD2), np.float32)
    bitsT = np.zeros((M, 56, NPAD2), np.float32)
    brel = np.full((M, 128, NB), 200.0, np.float32)
    pidx = np.zeros((M, 128, NB), np.int32)

    for c in range(M):
        sl = slice(c * NPC, (c + 1) * NPC)
        pos = pos_all[sl]
        x0row[c, 0, pos] = x[sl][:, 0].astype(np.float32)
        bitsT[c][:, pos] = _bits_rows(x[sl][:, 1:8]).T
        bc = batch[sl]
        for b in range(NB):
            lanes = np.where(pos // 128 == b)[0]
            lane_of = pos[lanes] % 128
            gb = int(bc[lanes].min()) if len(lanes) else 0
            assert len(lanes) == 0 or int(bc[lanes].max()) - gb < 128
            brel[c, lane_of, b] = bc[lanes] - gb
            pidx[c, :, b] = gb + np.arange(128)
            mask7[c, :nfill[c, b], b] = 1.0
        # edges of this core grouped by target block
        em = (tgt >= c * NPC) & (tgt < (c + 1) * NPC)
        et, es, ep = tgt[em] - c * NPC, src[em], pair[em]
        epos = pos[et]
        eb = epos // 128
        order = np.argsort(eb, kind="stable")
        es, ep, epos, eb = es[order], ep[order], epos[order], eb[order]
        starts = np.searchsorted(eb, np.arange(NB + 1))
        for g in range(NSB):
            for j in range(SBW):
                b = g * SBW + j
                e0, e1 = starts[b], starts[b + 1]
                K = int(Kvec[j])
                assert e1 - e0 <= K * 128
                for k in range(K):
                    lo = e0 + k * 128
                    hi = min(e1, lo + 128)
                    mlen = max(hi - lo, 0)
                    s = sbase[j] + k
                    c0 = g * 3 * NSLOT
                    if mlen > 0:
                        idx3[c, :mlen, c0 + s] = gpad[es[lo:hi]]
                        idx3[c, :mlen, c0 + NSLOT + s] = epos[lo:hi]
                        idx3[c, :mlen, c0 + 2 * NSLOT + s] = ep[lo:hi]
                        trel[c, :mlen, g * NSLOT + s] = \
                            (epos[lo:hi] % 128).astype(np.float32)

    in_maps = []
    for c in range(M):
        im = dict(W)
        im["idx3"] = idx3[c]
        im["trel"] = trel[c]
        im["mask7"] = mask7[c]
        im["x0row"] = x0row[c]
        im["bitsT"] = bitsT[c]
        im["brel"] = brel[c]
        im["pidx"] = pidx[c]
        im["lfT"] = lfT_h[c]
        im["rcg"] = rcg_h[c]
        in_maps.append(im)

    spec = {"Kvec": list(Kvec)}
    return in_maps, spec, pos_all


def cache_key(spec):
    return tuple(spec["Kvec"])


# ------------------------------------------------------------------ build
def build(spec):
    Kvec = list(spec["Kvec"])
    NSLOT = int(sum(Kvec))
    sbase = np.concatenate([[0], np.cumsum(Kvec)]).astype(int)
    NSL = NSLOT + SBW          # edge + self slots per superblock

    nc = bacc.Bacc("TRN2", target_bir_lowering=False, debug=False,
                   enable_asserts=False, num_devices=M)

    def din(name, shape, dt=F32):
        return nc.dram_tensor(name, list(shape), dt, kind="ExternalInput").ap()

    t_idx3 = din("idx3", [128, NSB * 3 * NSLOT], I32)
    t_trel = din("trel", [128, NSB * NSLOT])
    t_mask7 = din("mask7", [128, NB])
    t_x0row = din("x0row", [1, NPAD2])
    t_bitsT = din("bitsT", [56, NPAD2])
    t_brel = din("brel", [128, NB])
    t_pidx = din("pidx", [128, NB], I32)
    t_lfT = din("lfT", [128, NPAD2], BF16)
    t_rcg = din("rcg", [128, GPC // 128])
    t_eetp = din("eetab_pairs", [L, NPAIR, 256])
    t_wcat = din("wcat", [L, 2, 128, 512], BF16)
    t_xlrb = din("xlr_b", [L, 128, 512])
    t_lew = din("lew", [L, 128, 256], BF16)
    t_att = din("att_rep", [L, 128, 256])
    t_cvb = din("convb_rep", [L, 128, 256])
    t_bng = din("bng", [L, 1, 256])
    t_bnb = din("bnb", [L, 1, 256])
    t_aemb = din("aemb_pad", [128, 128])
    t_alw = din("alw", [56, 128])
    t_albc = din("alb_col", [128, 1])
    t_iota = din("iota", [128, 128])
    t_w1 = din("w1", [256, 1024], BF16)
    t_w2 = din("w2", [1024, 1024], BF16)
    t_w3 = din("w3", [1024, 512], BF16)
    t_w4 = din("w4", [512, NCLS], BF16)
    t_b1 = din("b1_rep", [128, 1024])
    t_b2 = din("b2_rep", [128, 1024])
    t_b3 = din("b3_rep", [128, 512])
    t_b4 = din("b4_rep", [128, NCLS])

    out_y = nc.dram_tensor("out_y", [GPC, NCLS], F32, kind="ExternalOutput").ap()

    with tile.TileContext(nc) as tc:
        with (
            tc.tile_pool(name="cst", bufs=1) as cst,
            tc.tile_pool(name="dram", bufs=1, space="DRAM") as dram,
        ):
            d_xl = dram.tile([NPAD2, 256], F32)
            d_xr = dram.tile([NPAD2, 256], F32)
            d_xl_alls = [dram.tile([M * NPAD, 256], F32, addr_space="Shared",
                                   name=f"xla{l}") for l in range(L)]
            d_eetabs = [dram.tile([NPAIR + NPAD2, 256], F32, name=f"eet{l}")
                        for l in range(L)]
            d_st = dram.tile([128, NSB * NSLOT * 128], F32)
            d_out = dram.tile([NPAD, 256], F32)
            d_pool = dram.tile([BPAD, 256], F32)
            d_pool_rs = dram.tile([GPC, 256], F32, name="poolrs")
            d_sin = dram.tile([1, 512], F32)
            d_souts = [dram.tile([1, 512], F32, addr_space="Shared",
                                 name=f"so{l}") for l in range(L)]

            for l in range(L):
                nc.sync.dma_start(d_eetabs[l][:NPAIR, :], t_eetp[l])

            # ---------------- persistent constants ----------------
            iota_f = cst.tile([128, 128], F32)
            nc.sync.dma_start(iota_f[:], t_iota[:])
            iotac = cst.tile([128, 1], F32)
            nc.sync.dma_start(iotac[:], t_iota[:].rearrange("a b -> b a")[:, :1])
            ones1 = cst.tile([1, 128], F32)
            nc.any.memset(ones1[:], 1.0)
            onesc = cst.tile([128, 1], F32)
            nc.any.memset(onesc[:], 1.0)
            wcat_sb = []
            for l in range(L):
                row = []
                for h in range(2):
                    w = cst.tile([128, 512], BF16, name=f"wc{l}{h}")
                    nc.sync.dma_start(w[:], t_wcat[l, h])
                    row.append(w)
                wcat_sb.append(row)
            xlrb_sb = cst.tile([128, L * 512], F32)
            lew_sb = cst.tile([128, L * 256], BF16)
            att_sb = cst.tile([128, L * 256], F32)
            cvb_sb = cst.tile([128, L * 256], F32)
            for l in range(L):
                nc.sync.dma_start(xlrb_sb[:, l * 512:(l + 1) * 512], t_xlrb[l])
                nc.sync.dma_start(lew_sb[:, l * 256:(l + 1) * 256], t_lew[l])
                nc.sync.dma_start(att_sb[:, l * 256:(l + 1) * 256], t_att[l])
                nc.sync.dma_start(cvb_sb[:, l * 256:(l + 1) * 256], t_cvb[l])
            aemb_sb = cst.tile([128, 128], F32)
            nc.sync.dma_start(aemb_sb[:], t_aemb[:])
            alw_sb = cst.tile([56, 128], F32)
            nc.sync.dma_start(alw_sb[:], t_alw[:])
            albc = cst.tile([128, 1], F32)
            nc.sync.dma_start(albc[:], t_albc[:])

            # ---------------- phase A: st one-hot precompute ----------
            with tc.tile_pool(name="sbA", bufs=1) as sbA:
                with tc.For_i(0, NSB, 1) as g:
                    trels = sbA.tile([128, NSLOT], F32, tag="trels")
                    nc.sync.dma_start(trels[:], t_trel[:, ts(g, NSLOT)])
                    stw = sbA.tile([128, NSLOT * 128], F32, tag="stw")
                    for s in range(NSLOT):
                        nc.vector.tensor_scalar(
                            out=stw[:, s * 128:(s + 1) * 128], in0=iota_f[:],
                            scalar1=trels[:, s:s + 1], scalar2=None,
                            op0=ALU.is_equal)
                    nc.sync.dma_start(d_st[:, ts(g, NSLOT * 128)], stw[:])

            # ---------------- phase B: featurize -> layer-0 tables ----------
            with (
                tc.tile_pool(name="psB0", bufs=1, space="PSUM") as psB,
                tc.tile_pool(name="sbB0", bufs=1) as sbB,
            ):
                with tc.For_i(0, NB2 // 4, 1) as gf:
                    x0s = sbB.tile([1, 512], F32, tag="x0s")
                    nc.sync.dma_start(x0s[:], t_x0row[:, ts(gf, 512)])
                    bits = sbB.tile([56, 512], F32, tag="bits")
                    nc.sync.dma_start(bits[:], t_bitsT[:, ts(gf, 512)])
                    lfs = sbB.tile([128, 512], BF16, tag="lfs")
                    nc.sync.dma_start(lfs[:], t_lfT[:, ts(gf, 512)])
                    rep_ps = psB.tile([128, 512], F32, space="PSUM", tag="rep")
                    nc.tensor.matmul(rep_ps[:], lhsT=ones1[:], rhs=x0s[:],
                                     start=True, stop=True)
                    oh = sbB.tile([128, 512], F32, tag="oh")
                    nc.vector.tensor_scalar(out=oh[:], in0=rep_ps[:],
                                            scalar1=iotac[:, :1], scalar2=None,
                                            op0=ALU.is_equal)
                    top_ps = psB.tile([128, 512], F32, space="PSUM", tag="top")
                    nc.tensor.matmul(top_ps[:], lhsT=aemb_sb[:], rhs=oh[:],
                                     start=True, stop=True)
                    bot_ps = psB.tile([128, 512], F32, space="PSUM", tag="bot")
                    nc.tensor.matmul(bot_ps[:], lhsT=alw_sb[:], rhs=bits[:],
                                     start=True, stop=True)
                    topb = sbB.tile([128, 512], BF16, tag="topb")
                    nc.vector.tensor_scalar(out=topb[:], in0=top_ps[:],
                                            scalar1=1.0, scalar2=None,
                                            op0=ALU.mult)
                    botb = sbB.tile([128, 512], BF16, tag="botb")
                    nc.vector.tensor_scalar(out=botb[:], in0=bot_ps[:],
                                            scalar1=albc[:, :1], scalar2=None,
                                            op0=ALU.add)
                    xlrw = sbB.tile([128, 4 * 512], F32, tag="xlrw")
                    seew = sbB.tile([128, 4 * 256], F32, tag="seew")
                    for i in range(4):
                        xlr_ps = psB.tile([128, 512], F32, space="PSUM",
                                          tag="xlr")
                        nc.tensor.matmul(xlr_ps[:],
                                         lhsT=topb[:, i * 128:(i + 1) * 128],
                                         rhs=wcat_sb[0][0][:], start=True,
                                         stop=False)
                        nc.tensor.matmul(xlr_ps[:],
                                         lhsT=botb[:, i * 128:(i + 1) * 128],
                                         rhs=wcat_sb[0][1][:], start=False,
                                         stop=True)
                        see_ps = psB.tile([128, 256], F32, space="PSUM",
                                          tag="see")
                        nc.tensor.matmul(see_ps[:],
                                         lhsT=lfs[:, i * 128:(i + 1) * 128],
                                         rhs=lew_sb[:, :256], start=True,
                                         stop=True)
                        nc.vector.tensor_tensor(
                            out=xlrw[:, i * 512:(i + 1) * 512], in0=xlr_ps[:],
                            in1=xlrb_sb[:, :512], op=ALU.add)
                        nc.vector.tensor_copy(seew[:, i * 256:(i + 1) * 256],
                                              see_ps[:])
                    nc.sync.dma_start(
                        d_xl[ts(gf, 512), :].rearrange("(b p) c -> p b c",
                                                       p=128),
                        xlrw[:].rearrange("p (b c) -> p b c", b=4)[:, :, 0:256])
                    nc.sync.dma_start(
                        d_xr[ts(gf, 512), :].rearrange("(b p) c -> p b c",
                                                       p=128),
                        xlrw[:].rearrange("p (b c) -> p b c",
                                          b=4)[:, :, 256:512])
                    nc.sync.dma_start(
                        d_eetabs[0][ds(gf * 512 + NPAIR, 512), :]
                        .rearrange("(b p) c -> p b c", p=128),
                        seew[:].rearrange("p (b c) -> p b c", b=4))

            # ---------------- conv layers ----------------
            for l in range(L):
                H = HEADS if l == 0 else 1
                Wyp = 256 + H
                CD = 256 // H
                lsl = slice(l * 256, (l + 1) * 256)
                nc.gpsimd.collective_compute(
                    "AllGather", ALU.bypass, ins=[d_xl[:NPAD, :]],
                    outs=[d_xl_alls[l].opt()], replica_groups=[list(range(M))])

                with (
                    tc.tile_pool(name=f"psE{l}", bufs=1, space="PSUM") as psE,
                    tc.tile_pool(name=f"sbE{l}", bufs=1) as sbE,
                ):
                    stats_acc = sbE.tile([1, 512], F32)
                    nc.any.memset(stats_acc[:], 0.0)
                    with tc.For_i(0, NSB, 1) as g:
                        idxs = sbE.tile([128, 3 * NSLOT], I32, tag="idxs")
                        nc.sync.dma_start(idxs[:], t_idx3[:, ts(g, 3 * NSLOT)])
                        mks = sbE.tile([128, SBW], F32, tag="mks")
                        nc.sync.dma_start(mks[:], t_mask7[:, ts(g, SBW)])
                        xls = sbE.tile([128, NSL * 256], F32, tag="xls")
                        xrg = sbE.tile([128, NSL * 256], F32, tag="xrg")
                        v = sbE.tile([128, NSL * 256], F32, tag="v")
                        for s in range(NSLOT):
                            nc.gpsimd.indirect_dma_start(
                                out=xls[:, s * 256:(s + 1) * 256],
                                out_offset=None, in_=d_xl_alls[l].opt(),
                                in_offset=bass.IndirectOffsetOnAxis(
                                    ap=idxs[:, s:s + 1], axis=0))
                            nc.gpsimd.indirect_dma_start(
                                out=xrg[:, s * 256:(s + 1) * 256],
                                out_offset=None, in_=d_xr.opt(),
                                in_offset=bass.IndirectOffsetOnAxis(
                                    ap=idxs[:, NSLOT + s:NSLOT + s + 1],
                                    axis=0))
                            nc.gpsimd.indirect_dma_start(
                                out=v[:, s * 256:(s + 1) * 256],
                                out_offset=None, in_=d_eetabs[l].opt(),
                                in_offset=bass.IndirectOffsetOnAxis(
                                    ap=idxs[:, 2 * NSLOT + s:2 * NSLOT + s + 1],
                                    axis=0))
                        selfsl = slice(NSLOT * 256, NSL * 256)
                        nc.sync.dma_start(
                            xls[:, selfsl].rearrange("p (b c) -> p b c", b=SBW),
                            d_xl[ts(g, 896), :].rearrange("(b p) c -> p b c",
                                                          p=128))
                        nc.sync.dma_start(
                            xrg[:, selfsl].rearrange("p (b c) -> p b c", b=SBW),
                            d_xr[ts(g, 896), :].rearrange("(b p) c -> p b c",
                                                          p=128))
                        nc.sync.dma_start(
                            v[:, selfsl].rearrange("p (b c) -> p b c", b=SBW),
                            d_eetabs[l][ds(g * 896 + NPAIR, 896), :]
                            .rearrange("(b p) c -> p b c", p=128))
                        wv = slice(0, NSL * 256)
                        nc.vector.tensor_tensor(out=v[:, wv], in0=v[:, wv],
                                                in1=xls[:, wv], op=ALU.add)
                        nc.vector.tensor_tensor(out=v[:, wv], in0=v[:, wv],
                                                in1=xrg[:, wv], op=ALU.add)
                        nc.vector.scalar_tensor_tensor(
                            out=v[:, wv], in0=v[:, wv], scalar=0.2,
                            in1=v[:, wv], op0=ALU.mult, op1=ALU.max)
                        am = sbE.tile([128, NSL * 256], F32, tag="xrg")
                        nc.vector.tensor_tensor(
                            out=am[:, wv].rearrange("p (s c) -> p s c", s=NSL),
                            in0=v[:, wv].rearrange("p (s c) -> p s c", s=NSL),
                            in1=att_sb[:, lsl].rearrange("p (u c) -> p u c",
                                                         u=1)
                                .broadcast_to([128, NSL, 256]), op=ALU.mult)
                        ypw = sbE.tile([128, NSL * 264], F32, tag="ypw")
                        yv = ypw[:, :NSL * Wyp].rearrange("p (s w) -> p s w",
                                                          w=Wyp)
                        nc.vector.reduce_sum(
                            yv[:, :, 256:Wyp],
                            am[:, wv].rearrange("p (s h c) -> p s h c",
                                                s=NSL, h=H), axis=AX)
                        nc.scalar.activation(yv[:, :, 256:Wyp],
                                             yv[:, :, 256:Wyp], ACTF.Exp)
                        nc.vector.tensor_tensor(
                            out=yv[:, :, 0:256].rearrange(
                                "p s (h c) -> p s h c", h=H),
                            in0=xls[:, wv].rearrange("p (s h c) -> p s h c",
                                                     s=NSL, h=H),
                            in1=yv[:, :, 256:Wyp].rearrange(
                                "p s (h u) -> p s h u", u=1)
                                .broadcast_to([128, NSL, H, CD]), op=ALU.mult)
                        stw = sbE.tile([128, NSLOT * 128], F32, tag="stw")
                        nc.sync.dma_start(stw[:], d_st[:, ts(g, NSLOT * 128)])
                        ndps = [psE.tile([128, Wyp], F32, space="PSUM",
                                         tag=f"nd{j}", name=f"ndps{j}")
                                for j in range(SBW)]
                        for j in range(SBW):
                            K = int(Kvec[j])
                            for k in range(K):
                                s = int(sbase[j]) + k
                                nc.tensor.matmul(
                                    ndps[j][:],
                                    lhsT=stw[:, s * 128:(s + 1) * 128],
                                    rhs=ypw[:, s * Wyp:(s + 1) * Wyp],
                                    start=(k == 0), stop=(k == K - 1))
                        ndw = sbE.tile([128, SBW * 264], F32, tag="ndw")
                        for j in range(SBW):
                            ssl = slice((NSLOT + j) * Wyp,
                                        (NSLOT + j + 1) * Wyp)
                            osl = slice(j * Wyp, (j + 1) * Wyp)
                            nc.vector.tensor_tensor(
                                out=ndw[:, osl], in0=ndps[j][:],
                                in1=ypw[:, ssl], op=ALU.add)
                        nv = ndw[:, :SBW * Wyp].rearrange("p (s w) -> p s w",
                                                          w=Wyp)
                        rdn = sbE.tile([128, SBW * 8], F32, tag="rdn")
                        nc.vector.reciprocal(rdn[:, :SBW * H],
                                             nv[:, :, 256:Wyp])
                        sqi = sbE.tile([128, SBW * 512], F32, tag="sqi")
                        sv = sqi[:].rearrange("p (s w) -> p s w", w=512)
                        xv = sv[:, :, 0:256]
                        nc.vector.tensor_tensor(
                            out=xv.rearrange("p s (h c) -> p s h c", h=H),
                            in0=nv[:, :, 0:256].rearrange(
                                "p s (h c) -> p s h c", h=H),
                            in1=rdn[:, :SBW * H].rearrange(
                                "p (s h u) -> p s h u", s=SBW, u=1)
                                .broadcast_to([128, SBW, H, CD]), op=ALU.mult)
                        nc.vector.tensor_tensor(
                            out=xv, in0=xv,
                            in1=cvb_sb[:, lsl].rearrange("p (u c) -> p u c",
                                                         u=1)
                                .broadcast_to([128, SBW, 256]), op=ALU.add)
                        nc.vector.tensor_tensor(
                            out=xv, in0=xv,
                            in1=mks[:].rearrange("p (s u) -> p s u", u=1)
                                .broadcast_to([128, SBW, 256]), op=ALU.mult)
                        nc.vector.tensor_tensor(
                            out=sv[:, :, 256:512], in0=xv, in1=xv,
                            op=ALU.mult)
                        stats_ps = psE.tile([1, 512], F32, space="PSUM",
                                            tag="stats")
                        for j in range(SBW):
                            nc.tensor.matmul(
                                stats_ps[:], lhsT=onesc[:],
                                rhs=sqi[:, j * 512:(j + 1) * 512],
                                start=(j == 0), stop=(j == SBW - 1))
                        nc.vector.tensor_tensor(out=stats_acc[:],
                                                in0=stats_acc[:],
                                                in1=stats_ps[:], op=ALU.add)
                        nc.sync.dma_start(
                            d_out[ts(g, 896), :].rearrange("(b p) c -> p b c",
                                                           p=128),
                            sv[:, :, 0:256])
                    nc.sync.dma_start(d_sin[:], stats_acc[:])

                nc.gpsimd.collective_compute(
                    "AllReduce", ALU.add, ins=[d_sin.opt()],
                    outs=[d_souts[l].opt()], replica_groups=[list(range(M))])

                # ---- BN coeffs + next tables / pooling ----
                with (
                    tc.tile_pool(name=f"psP{l}", bufs=1, space="PSUM") as psP,
                    tc.tile_pool(name=f"sbP{l}", bufs=1) as sbP,
                ):
                    stg = sbP.tile([1, 512], F32)
                    nc.sync.dma_start(stg[:], d_souts[l].opt())
                    rowAB = sbP.tile([1, 512], F32)
                    mu = sbP.tile([1, 256], F32)
                    nc.vector.tensor_scalar(out=mu[:], in0=stg[:, :256],
                                            scalar1=1.0 / N, scalar2=None,
                                            op0=ALU.mult)
                    ex2 = sbP.tile([1, 256], F32)
                    nc.vector.tensor_scalar(out=ex2[:], in0=stg[:, 256:],
                                            scalar1=1.0 / N, scalar2=None,
                                            op0=ALU.mult)
                    var = sbP.tile([1, 256], F32)
                    nc.vector.tensor_tensor(out=var[:], in0=mu[:], in1=mu[:],
                                            op=ALU.mult)
                    nc.vector.tensor_tensor(out=var[:], in0=ex2[:], in1=var[:],
                                            op=ALU.subtract)
                    nc.vector.tensor_scalar(out=var[:], in0=var[:],
                                            scalar1=1e-5, scalar2=None,
                                            op0=ALU.add)
                    sd = sbP.tile([1, 256], F32)
                    nc.scalar.activation(sd[:], var[:], ACTF.Sqrt)
                    rstd = sbP.tile([1, 256], F32)
                    nc.vector.reciprocal(rstd[:], sd[:])
                    bng = sbP.tile([1, 256], F32)
                    nc.sync.dma_start(bng[:], t_bng[l])
                    bnb = sbP.tile([1, 256], F32)
                    nc.sync.dma_start(bnb[:], t_bnb[l])
                    nc.vector.tensor_tensor(out=rowAB[:, :256], in0=rstd[:],
                                            in1=bng[:], op=ALU.mult)
                    t3 = sbP.tile([1, 256], F32)
                    nc.vector.tensor_tensor(out=t3[:], in0=mu[:],
                                            in1=rowAB[:, :256], op=ALU.mult)
                    nc.vector.tensor_tensor(out=rowAB[:, 256:], in0=bnb[:],
                                            in1=t3[:], op=ALU.subtract)
                    rab_ps = psP.tile([128, 512], F32, space="PSUM", tag="rab")
                    nc.tensor.matmul(rab_ps[:], lhsT=ones1[:], rhs=rowAB[:],
                                     start=True, stop=True)
                    rab = sbP.tile([128, 512], F32)
                    nc.vector.tensor_copy(rab[:], rab_ps[:])

                    if l < L - 1:
                        # ---- pass B: next-layer tables ----
                        with tc.For_i(0, NSB, 1) as g:
                            xnw = sbP.tile([128, SBW * 256], F32, tag="xnw")
                            nc.sync.dma_start(
                                xnw[:].rearrange("p (b c) -> p b c", b=SBW),
                                d_out[ts(g, 896), :].rearrange(
                                    "(b p) c -> p b c", p=128))
                            nc.vector.tensor_tensor(
                                out=xnw[:].rearrange("p (s c) -> p s c", s=SBW),
                                in0=xnw[:].rearrange("p (s c) -> p s c", s=SBW),
                                in1=rab[:, :256].rearrange("p (u c) -> p u c",
                                                           u=1)
                                    .broadcast_to([128, SBW, 256]),
                                op=ALU.mult)
                            nc.vector.tensor_tensor(
                                out=xnw[:].rearrange("p (s c) -> p s c", s=SBW),
                                in0=xnw[:].rearrange("p (s c) -> p s c", s=SBW),
                                in1=rab[:, 256:].rearrange("p (u c) -> p u c",
                                                           u=1)
                                    .broadcast_to([128, SBW, 256]),
                                op=ALU.add)
                            xnb = sbP.tile([128, SBW * 256], BF16, tag="xnb")
                            nc.vector.scalar_tensor_tensor(
                                out=xnb[:], in0=xnw[:], scalar=0.01,
                                in1=xnw[:], op0=ALU.mult, op1=ALU.max)
                            lfs = sbP.tile([128, 896], BF16, tag="lfs")
                            nc.sync.dma_start(lfs[:], t_lfT[:, ts(g, 896)])
                            xlrw = sbP.tile([128, SBW * 512], F32, tag="xlrw")
                            seew = sbP.tile([128, SBW * 256], F32, tag="seew")
                            for j in range(SBW):
                                xnT = sbP.tile([128, 256], BF16, tag="xnT")
                                for h in range(2):
                                    nc.sync.dma_start_transpose(
                                        xnT[:, h * 128:(h + 1) * 128],
                                        xnb[:, j * 256 + h * 128:
                                            j * 256 + (h + 1) * 128])
                                xlr_ps = psP.tile([128, 512], F32,
                                                  space="PSUM", tag="xlr")
                                for h in range(2):
                                    nc.tensor.matmul(
                                        xlr_ps[:],
                                        lhsT=xnT[:, h * 128:(h + 1) * 128],
                                        rhs=wcat_sb[l + 1][h][:],
                                        start=(h == 0), stop=(h == 1))
                                see_ps = psP.tile([128, 256], F32,
                                                  space="PSUM", tag="see")
                                nc.tensor.matmul(
                                    see_ps[:],
                                    lhsT=lfs[:, j * 128:(j + 1) * 128],
                                    rhs=lew_sb[:, (l + 1) * 256:(l + 2) * 256],
                                    start=True, stop=True)
                                nc.vector.tensor_tensor(
                                    out=xlrw[:, j * 512:(j + 1) * 512],
                                    in0=xlr_ps[:],
                                    in1=xlrb_sb[:, (l + 1) * 512:(l + 2) * 512],
                                    op=ALU.add)
                                nc.vector.tensor_copy(
                                    seew[:, j * 256:(j + 1) * 256], see_ps[:])
                            nc.sync.dma_start(
                                d_xl[ts(g, 896), :].rearrange(
                                    "(b p) c -> p b c", p=128),
                                xlrw[:].rearrange("p (b c) -> p b c",
                                                  b=SBW)[:, :, 0:256])
                            nc.sync.dma_start(
                                d_xr[ts(g, 896), :].rearrange(
                                    "(b p) c -> p b c", p=128),
                                xlrw[:].rearrange("p (b c) -> p b c",
                                                  b=SBW)[:, :, 256:512])
                            nc.sync.dma_start(
                                d_eetabs[l + 1][ds(g * 896 + NPAIR, 896), :]
                                .rearrange("(b p) c -> p b c", p=128),
                                seew[:].rearrange("p (b c) -> p b c", b=SBW))
                    else:
                        # ---- pooling ----
                        zer = sbP.tile([128, 256], F32, tag="zer")
                        nc.any.memset(zer[:], 0.0)
                        with tc.For_i(0, BPAD // 128, 1) as zi:
                            nc.sync.dma_start(d_pool[ts(zi, 128), :], zer[:])
                        with tc.For_i(0, NSB, 1) as g:
                            xnw = sbP.tile([128, SBW * 256], F32, tag="xnw")
                            nc.sync.dma_start(
                                xnw[:].rearrange("p (b c) -> p b c", b=SBW),
                                d_out[ts(g, 896), :].rearrange(
                                    "(b p) c -> p b c", p=128))
                            nc.vector.tensor_tensor(
                                out=xnw[:].rearrange("p (s c) -> p s c", s=SBW),
                                in0=xnw[:].rearrange("p (s c) -> p s c", s=SBW),
                                in1=rab[:, :256].rearrange("p (u c) -> p u c",
                                                           u=1)
                                    .broadcast_to([128, SBW, 256]),
                                op=ALU.mult)
                            nc.vector.tensor_tensor(
                                out=xnw[:].rearrange("p (s c) -> p s c", s=SBW),
                                in0=xnw[:].rearrange("p (s c) -> p s c", s=SBW),
                                in1=rab[:, 256:].rearrange("p (u c) -> p u c",
                                                           u=1)
                                    .broadcast_to([128, SBW, 256]),
                                op=ALU.add)
                            brs = sbP.tile([128, SBW], F32, tag="brs")
                            nc.sync.dma_start(brs[:], t_brel[:, ts(g, SBW)])
                            pis = sbP.tile([128, SBW], I32, tag="pis")
                            nc.sync.dma_start(pis[:], t_pidx[:, ts(g, SBW)])
                            for j in range(SBW):
                                ohp = sbP.tile([128, 128], F32, tag="ohp")
                                nc.vector.tensor_scalar(
                                    out=ohp[:], in0=iota_f[:],
                                    scalar1=brs[:, j:j + 1], scalar2=None,
                                    op0=ALU.is_equal)
                                pool_ps = psP.tile([128, 256], F32,
                                                   space="PSUM", tag="pool")
                                nc.tensor.matmul(
                                    pool_ps[:], lhsT=ohp[:],
                                    rhs=xnw[:, j * 256:(j + 1) * 256],
                                    start=True, stop=True)
                                pool_sb = sbP.tile([128, 256], F32,
                                                   tag="poolsb")
                                nc.vector.tensor_copy(pool_sb[:], pool_ps[:])
                                nc.gpsimd.indirect_dma_start(
                                    out=d_pool.opt(), in_=pool_sb[:],
                                    in_offset=None,
                                    out_offset=bass.IndirectOffsetOnAxis(
                                        ap=pis[:, j:j + 1], axis=0),
                                    compute_op=ALU.add)

            nc.gpsimd.collective_compute(
                "ReduceScatter", ALU.add, ins=[d_pool[:B, :]],
                outs=[d_pool_rs.opt()], replica_groups=[list(range(M))])

            # ---------------- MLP ----------------
            with (
                tc.tile_pool(name="psM", bufs=1, space="PSUM") as psM,
                tc.tile_pool(name="sbM", bufs=1) as sbM,
                tc.tile_pool(name="wM", bufs=1) as wM,
            ):
                w1s = wM.tile([128, 2 * 1024], BF16)
                for i in range(2):
                    nc.sync.dma_start(w1s[:, i * 1024:(i + 1) * 1024],
                                      t_w1[i * 128:(i + 1) * 128, :])
                w2s = wM.tile([128, 8 * 1024], BF16)
                for i in range(8):
                    nc.sync.dma_start(w2s[:, i * 1024:(i + 1) * 1024],
                                      t_w2[i * 128:(i + 1) * 128, :])
                w3s = wM.tile([128, 8 * 512], BF16)
                for i in range(8):
                    nc.sync.dma_start(w3s[:, i * 512:(i + 1) * 512],
                                      t_w3[i * 128:(i + 1) * 128, :])
                w4s = wM.tile([128, 4 * NCLS], BF16)
                for i in range(4):
                    nc.sync.dma_start(w4s[:, i * NCLS:(i + 1) * NCLS],
                                      t_w4[i * 128:(i + 1) * 128, :])
                b1s = wM.tile([128, 1024], F32)
                nc.sync.dma_start(b1s[:], t_b1[:])
                b2s = wM.tile([128, 1024], F32)
                nc.sync.dma_start(b2s[:], t_b2[:])
                b3s = wM.tile([128, 512], F32)
                nc.sync.dma_start(b3s[:], t_b3[:])
                b4s = wM.tile([128, NCLS], F32)
                nc.sync.dma_start(b4s[:], t_b4[:])

                def ffn(xT, xwidth, ws, wwidth, bs, tagp):
                    nin = xwidth // 128
                    nps = (wwidth + 511) // 512
                    hf = sbM.tile([128, wwidth], F32, tag=f"hf{tagp}")
                    for np_ in range(nps):
                        wlo = np_ * 512
                        whi = min(wwidth, wlo + 512)
                        hp = psM.tile([128, 512], F32, space="PSUM",
                                      tag=f"hp{np_}")
                        for kk in range(nin):
                            nc.tensor.matmul(
                                hp[:, :whi - wlo],
                                lhsT=xT[:, kk * 128:(kk + 1) * 128],
                                rhs=ws[:, kk * wwidth + wlo:kk * wwidth + whi],
                                start=(kk == 0), stop=(kk == nin - 1))
                        nc.vector.tensor_tensor(out=hf[:, wlo:whi],
                                                in0=hp[:, :whi - wlo],
                                                in1=bs[:, wlo:whi], op=ALU.add)
                    return hf

                def transp(hf, width, dorelu=True):
                    hb = sbM.tile([128, width], BF16, tag=f"hb{width}")
                    if dorelu:
                        nc.scalar.activation(hb[:], hf[:], ACTF.Relu)
                    else:
                        nc.vector.tensor_copy(hb[:], hf[:])
                    hT = sbM.tile([128, width], BF16, tag=f"hT{width}")
                    for i in range(width // 128):
                        nc.sync.dma_start_transpose(
                            hT[:, i * 128:(i + 1) * 128],
                            hb[:, i * 128:(i + 1) * 128])
                    return hT

                with tc.For_i(0, GPC // 128, 1) as mi:
                    pc = sbM.tile([128, 256], F32, tag="pc")
                    nc.sync.dma_start(pc[:], d_pool_rs[ts(mi, 128), :])
                    rcgs = sbM.tile([128, 1], F32, tag="rcgs")
                    nc.sync.dma_start(rcgs[:], t_rcg[:, ts(mi, 1)])
                    g0 = sbM.tile([128, 256], F32, tag="g0")
                    nc.vector.tensor_scalar(out=g0[:], in0=pc[:],
                                            scalar1=rcgs[:, :1],
                                            scalar2=None, op0=ALU.mult)
                    gT = transp(g0, 256, dorelu=False)
                    h1 = ffn(gT, 256, w1s, 1024, b1s, "1")
                    h1T = transp(h1, 1024)
                    h2 = ffn(h1T, 1024, w2s, 1024, b2s, "2")
                    h2T = transp(h2, 1024)
                    h3 = ffn(h2T, 1024, w3s, 512, b3s, "3")
                    h3T = transp(h3, 512)
                    yp = psM.tile([128, NCLS], F32, space="PSUM", tag="yp")
                    for kk in range(4):
                        nc.tensor.matmul(yp[:],
                                         lhsT=h3T[:, kk * 128:(kk + 1) * 128],
                                         rhs=w4s[:, kk * NCLS:(kk + 1) * NCLS],
                                         start=(kk == 0), stop=(kk == 3))
                    yo = sbM.tile([128, NCLS], F32, tag="yo")
                    nc.vector.tensor_tensor(out=yo[:], in0=yp[:], in1=b4s[:],
                                            op=ALU.add)
                    nc.sync.dma_start(out_y[ts(mi, 128), :], yo[:])

    nc.compile()
    return nc


# ------------------------------------------------------------------ entry
def kernel(**inputs) -> np.ndarray:
    in_maps, spec, _ = host_prep(inputs)
    key = cache_key(spec)
    if key not in _cache:
        _cache[key] = build(spec)
    nc = _cache[key]
    res = run_bass_kernel_spmd(nc, in_maps, list(range(M)))
    return np.concatenate([res.results[c]["out_y"] for c in range(M)], axis=0)
